# revision 1
# baseline (speedup 1.0000x reference)
"""Trainium2 Bass kernel for hyperbolic linear-attention transformer layer.

Data-parallel over nodes (N=32768) across 8 NeuronCores. Per core:
  Phase A: k/v head projections (PE, fp32r), phi_k nonlinearity (DVE/ACT),
           per-core partial ktv = phi_k^T v accumulated in PSUM, partial
           sum(phi_k) accumulated on DVE.
  AllReduce of [ktv | sumk] partials (2.1 MB) across the 8 cores.
  Phase B: q projection, phi_q, denominator folded into a per-(head,node)
           scale, attn^T computed feature-major (so the final projection
           needs no transposes), fused v_map path (W_vm = v_map_w @ Wv
           precomputed on host), final projection + Lorentz lift.

All matmuls run as float32r (full PE rate at moving-dim>=256).
"""

import os
import numpy as np
import concourse.bass as bass
import concourse.tile as tile
from concourse import bacc, mybir
from concourse.bass_utils import run_bass_kernel_spmd

F32 = mybir.dt.float32
F32R = mybir.dt.float32r
AF = mybir.ActivationFunctionType
ALU = mybir.AluOpType

NCORES = 8
N = 32768
NCHUNK = N // NCORES          # 4096 nodes per core
H = 8
D = 256
HD = H * D                    # 2048
KC = 3                        # contraction chunks: 384 = 3*128 (257 used)
EPS = 1e-6

_CACHE = {}


def _build(reps=1):
    if reps in _CACHE:
        return _CACHE[reps]
    onecore = bool(os.environ.get("KT_ONECORE"))
    nc = bacc.Bacc("TRN2", target_bir_lowering=False, debug=False,
                   num_devices=1 if onecore else NCORES)

    xqT = nc.dram_tensor("xqT", [KC, 128, NCHUNK], F32R, kind="ExternalInput").ap()
    xsT = nc.dram_tensor("xsT", [KC, 128, NCHUNK], F32R, kind="ExternalInput").ap()
    wq = nc.dram_tensor("wq", [KC, 128, HD], F32R, kind="ExternalInput").ap()
    wk = nc.dram_tensor("wk", [KC, 128, HD], F32R, kind="ExternalInput").ap()
    wv = nc.dram_tensor("wv", [KC, 128, HD], F32R, kind="ExternalInput").ap()
    wvm = nc.dram_tensor("wvm", [KC, 128, HD], F32R, kind="ExternalInput").ap()
    fw = nc.dram_tensor("fw", [16, 128, D], F32R, kind="ExternalInput").ap()
    fbias = nc.dram_tensor("fbias", [1, D], F32R, kind="ExternalInput").ap()
    ones_r = nc.dram_tensor("ones_r", [1, 128], F32R, kind="ExternalInput").ap()
    ones_c = nc.dram_tensor("ones_c", [128, 8], F32R, kind="ExternalInput").ap()
    ind = nc.dram_tensor("ind", [128, 8, 8], F32R, kind="ExternalInput").ap()
    ind2 = nc.dram_tensor("ind2", [8, 8, 128], F32R, kind="ExternalInput").ap()
    zt = nc.dram_tensor("zt", [128, 16, 8], F32R, kind="ExternalInput").ap()
    cons = nc.dram_tensor("cons", [8, 1], F32, kind="ExternalInput").ap()
    out = nc.dram_tensor("out", [NCHUNK, 257], F32, kind="ExternalOutput").ap()

    with tile.TileContext(nc) as tc:
        _body(nc, tc, reps, xqT, xsT, wq, wk, wv, wvm, fw, fbias,
              ones_r, ones_c, ind, ind2, zt, cons, out)
    nc.compile()
    _CACHE[reps] = nc
    return nc


def _body(nc, tc, reps, xqT, xsT, wq, wk, wv, wvm, fw, fbias,
          ones_r, ones_c, ind, ind2, zt, cons, out):
    import contextlib
    stack = contextlib.ExitStack()
    with stack:
        cpool = stack.enter_context(tc.tile_pool(name="const", bufs=1))
        dpool = stack.enter_context(tc.tile_pool(name="dram", bufs=1, space="DRAM"))

        ones_r_sb = cpool.tile([1, 128], F32R)
        nc.sync.dma_start(ones_r_sb[:], ones_r[:])
        ones_c_sb = cpool.tile([128, 8], F32R)
        nc.sync.dma_start(ones_c_sb[:], ones_c[:])
        ind_sb = cpool.tile([128, 8, 8], F32R)
        nc.sync.dma_start(ind_sb[:], ind[:])
        ind2_sb = cpool.tile([8, 8, 128], F32R)
        nc.sync.dma_start(ind2_sb[:], ind2[:])
        fb_sb = cpool.tile([1, D], F32R)
        nc.sync.dma_start(fb_sb[:], fbias[:])
        eps_sb = cpool.tile([8, 1], F32)
        nc.sync.dma_start(eps_sb[:], cons[:])

        ar_in = dpool.tile([129, 4096], F32)
        ar_out = dpool.tile([129, 4096], F32)

        for rep in range(reps):
            if not os.environ.get("KT_SKIP_A"):
                _phase_a(nc, tc, xsT, wk, wv, ones_c_sb, ar_in)
            if os.environ.get("KT_ONECORE"):
                nc.sync.dma_start(ar_out[:], ar_in[:])
            else:
                nc.gpsimd.collective_compute(
                    "AllReduce", ALU.add,
                    replica_groups=[list(range(NCORES))],
                    ins=[ar_in.opt()], outs=[ar_out.opt()])
            if not os.environ.get("KT_SKIP_B"):
                _phase_b(nc, tc, xqT, xsT, wq, wvm, fw, fb_sb, ones_r_sb,
                         ind_sb, ind2_sb, zt, eps_sb, ar_out, out)
            else:
                obp = tc.tile_pool(name="oBtmp", bufs=1)
                with obp as ob:
                    o_sb = ob.tile([128, 257], F32)
                    nc.sync.dma_start(o_sb[:], ar_out[0:128, 0:257])
                    for t0_ in range(NCHUNK // 128):
                        nc.sync.dma_start(out[t0_ * 128:(t0_ + 1) * 128, :], o_sb[:])


def _phase_a(nc, tc, xsT, wk, wv, ones_c_sb, ar_in):
    import contextlib
    with contextlib.ExitStack() as st:
        wpool = st.enter_context(tc.tile_pool(name="wA", bufs=1))
        xp = st.enter_context(tc.tile_pool(name="xA", bufs=3))
        zp = st.enter_context(tc.tile_pool(name="zA", bufs=2))
        yp = st.enter_context(tc.tile_pool(name="yA", bufs=2))
        scrp = st.enter_context(tc.tile_pool(name="scrA", bufs=2))
        stp = st.enter_context(tc.tile_pool(name="stA", bufs=4))
        php = st.enter_context(tc.tile_pool(name="phA", bufs=2))
        vp = st.enter_context(tc.tile_pool(name="vA", bufs=2))
        drp = st.enter_context(tc.tile_pool(name="drA", bufs=2))
        pk = st.enter_context(tc.tile_pool(name="psAk", bufs=1, space="PSUM"))
        pp = st.enter_context(tc.tile_pool(name="psAp", bufs=3, space="PSUM"))
        psk = st.enter_context(tc.tile_pool(name="psAs", bufs=1, space="PSUM"))

        wk_sb = wpool.tile([128, KC, HD], F32R)
        nc.sync.dma_start(wk_sb[:], wk.rearrange("c p n -> p c n"))
        wv_sb = wpool.tile([128, KC, HD], F32R)
        nc.sync.dma_start(wv_sb[:], wv.rearrange("c p n -> p c n"))
        sumk_acc = wpool.tile([128, HD], F32R)

        ntiles = int(os.environ.get("KT_NTILES", NCHUNK // 128))
        for g in range(2):
            gofs = g * 1024
            ktv_ps = pk.tile([128, 4, 512], F32)
            for t in range(ntiles):
                xs_sb = xp.tile([128, KC, 128], F32R, tag="xs")
                nc.sync.dma_start(
                    xs_sb[:],
                    xsT[:, :, t * 128:(t + 1) * 128].rearrange("c p n -> p c n"))

                ks_ps = []
                vs_ps = []
                for blk in range(2):
                    kp_t = pp.tile([128, 512], F32, tag="projA")
                    for c in range(KC):
                        nc.tensor.matmul(
                            kp_t[:], lhsT=xs_sb[:, c],
                            rhs=wk_sb[:, c, gofs + blk * 512: gofs + blk * 512 + 512],
                            start=(c == 0), stop=(c == KC - 1))
                    ks_ps.append(kp_t)
                for blk in range(2):
                    vp_t = pp.tile([128, 512], F32, tag="projA")
                    for c in range(KC):
                        nc.tensor.matmul(
                            vp_t[:], lhsT=xs_sb[:, c],
                            rhs=wv_sb[:, c, gofs + blk * 512: gofs + blk * 512 + 512],
                            start=(c == 0), stop=(c == KC - 1))
                    vs_ps.append(vp_t)

                # z = relu(ks) + eps
                z = zp.tile([128, 1024], F32, tag="z")
                for blk in range(2):
                    nc.vector.tensor_scalar(
                        z[:, blk * 512:(blk + 1) * 512], ks_ps[blk][:],
                        0.0, EPS, ALU.max, ALU.add)
                # v copy to SBUF (frees psum quickly)
                v_sb = vp.tile([128, 1024], F32R, tag="v")
                for blk in range(2):
                    nc.scalar.copy(v_sb[:, blk * 512:(blk + 1) * 512], vs_ps[blk][:])

                # y = z^2 with per-head accumulated sums
                y = yp.tile([128, 1024], F32R, tag="y")
                sy = stp.tile([128, 4], F32, tag="sy")
                sy2 = stp.tile([128, 4], F32, tag="sy2")
                for hh in range(4):
                    sl = slice(hh * 256, hh * 256 + 256)
                    nc.scalar.activation(y[:, sl], z[:, sl], AF.Square,
                                         accum_out=sy[:, hh:hh + 1])
                for hh in range(4):
                    sl = slice(hh * 256, hh * 256 + 256)
                    scr = scrp.tile([128, 256], F32, tag="y2scr")
                    nc.scalar.activation(scr[:], y[:, sl].bitcast(F32), AF.Square,
                                         accum_out=sy2[:, hh:hh + 1])
                # factor = sqrt(sy / sy2)
                rec = stp.tile([128, 4], F32, tag="rec")
                nc.vector.reciprocal(rec[:], sy2[:])
                rat = stp.tile([128, 4], F32, tag="rat")
                nc.vector.tensor_mul(rat[:], sy[:], rec[:])
                fac = stp.tile([128, 4], F32, tag="fac")
                nc.scalar.activation(fac[:], rat[:], AF.Sqrt)

                phi = php.tile([128, 1024], F32R, tag="phi")
                for hh in range(4):
                    sl = slice(hh * 256, hh * 256 + 256)
                    nc.vector.tensor_scalar_mul(phi[:, sl], y[:, sl].bitcast(F32),
                                                fac[:, hh:hh + 1])
                # sumk accumulation
                dst = sumk_acc[:, gofs:gofs + 1024]
                if t == 0:
                    nc.scalar.copy(dst, phi[:].bitcast(F32))
                else:
                    nc.vector.tensor_add(dst, dst.bitcast(F32), phi[:].bitcast(F32))

                # ktv accumulation: ktv[h][m,d] += phi[:,h*256+mc*128]T v[:,h*256:]
                for hh in range(4 if not os.environ.get("KT_NO_KTV") else 0):
                    for mc in range(2):
                        nc.tensor.matmul(
                            ktv_ps[:, hh, mc * 256: mc * 256 + 256],
                            lhsT=phi[:, hh * 256 + mc * 128: hh * 256 + mc * 128 + 128],
                            rhs=v_sb[:, hh * 256: hh * 256 + 256],
                            start=(t == 0), stop=(t == ntiles - 1))

            # drain ktv partials for this head group
            if not os.environ.get("KT_NO_KTV"):
                ktv_sbt = drp.tile([128, 4, 512], F32, tag="ktvdr")
                for hh in range(4):
                    nc.scalar.copy(ktv_sbt[:, hh], ktv_ps[:, hh])
                nc.sync.dma_start(ar_in[0:128, g * 2048:(g + 1) * 2048],
                                  ktv_sbt[:].rearrange("p a b -> p (a b)"))
            # sumk partition-reduction for this group
            for blk in range(2 if not os.environ.get("KT_NO_SUMK") else 0):
                sps = psk.tile([8, 512], F32, tag="sumkps")
                nc.tensor.matmul(
                    sps[:], lhsT=ones_c_sb[:],
                    rhs=sumk_acc[:, gofs + blk * 512: gofs + blk * 512 + 512],
                    start=True, stop=True)
                srow = drp.tile([1, 512], F32, tag="srow")
                nc.scalar.copy(srow[:], sps[0:1, :])
                nc.sync.dma_start(
                    ar_in[128:129, gofs + blk * 512: gofs + blk * 512 + 512],
                    srow[:])


def _phase_b(nc, tc, xqT, xsT, wq, wvm, fw, fb_sb, ones_r_sb, ind_sb, ind2_sb,
             zt, eps_sb, ar_out, out):
    import contextlib
    with contextlib.ExitStack() as st:
        wpool = st.enter_context(tc.tile_pool(name="wB", bufs=1))
        xp = st.enter_context(tc.tile_pool(name="xB", bufs=2))
        zp = st.enter_context(tc.tile_pool(name="zB", bufs=3))
        yp = st.enter_context(tc.tile_pool(name="yB", bufs=17))
        y2p = st.enter_context(tc.tile_pool(name="y2B", bufs=3))
        stp = st.enter_context(tc.tile_pool(name="stB", bufs=2))
        php = st.enter_context(tc.tile_pool(name="phB", bufs=17))
        atp = st.enter_context(tc.tile_pool(name="atB", bufs=17))
        obp = st.enter_context(tc.tile_pool(name="oB", bufs=3))
        qp = st.enter_context(tc.tile_pool(name="psBq", bufs=2, space="PSUM"))
        sump = st.enter_context(tc.tile_pool(name="psBs", bufs=1, space="PSUM"))
        sbp = st.enter_context(tc.tile_pool(name="psBb", bufs=1, space="PSUM"))
        ap_ = st.enter_context(tc.tile_pool(name="psBa", bufs=2, space="PSUM"))
        op = st.enter_context(tc.tile_pool(name="psBo", bufs=1, space="PSUM"))

        wq_sb = wpool.tile([128, KC, HD], F32R)
        nc.sync.dma_start(wq_sb[:], wq.rearrange("c p n -> p c n"))
        wvm_sb = wpool.tile([128, KC, HD], F32R)
        nc.sync.dma_start(wvm_sb[:], wvm.rearrange("c p n -> p c n"))
        fw_sb = wpool.tile([128, 16, D], F32R)
        nc.sync.dma_start(fw_sb[:], fw.rearrange("c p n -> p c n"))
        # ktv (all-reduced) as lhsT chunks [m_loc, h, mc, dc, d_loc]
        ktv_sb = wpool.tile([128, H, 2, 2, 128], F32R)
        nc.gpsimd.dma_start(
            ktv_sb[:],
            ar_out[0:128, :].rearrange("p (h mc dc dl) -> p h mc dc dl",
                                       h=H, mc=2, dc=2))
        # sumk chunk columns: [128, 16, 8], chunk c -> column h(c)
        sumk_w = wpool.tile([128, 16, 8], F32R)
        nc.sync.dma_start(sumk_w[:], zt[:])
        for c in range(16):
            hh = c // 2
            nc.gpsimd.dma_start(
                sumk_w[:, c, hh:hh + 1],
                ar_out[128:129, c * 128:(c + 1) * 128].rearrange(
                    "r (p o) -> (r p) o", o=1))

        NST = 256                      # supertile node count
        nst = int(os.environ.get("KT_NST", NCHUNK // NST))
        for stx in range(nst):
            nofs = stx * NST
            xq_sb = xp.tile([128, KC, NST], F32R, tag="xq")
            nc.sync.dma_start(
                xq_sb[:], xqT[:, :, nofs:nofs + NST].rearrange("c p n -> p c n"))
            xs_sb = xp.tile([128, KC, NST], F32R, tag="xsB")
            nc.sync.dma_start(
                xs_sb[:], xsT[:, :, nofs:nofs + NST].rearrange("c p n -> p c n"))

            sums_ps = sump.tile([8, 3, NST], F32, tag="sums")
            ys = []
            for c in range(16):
                hh = c // 2
                q_ps = qp.tile([128, NST], F32, tag="qps")
                for kc in range(KC):
                    nc.tensor.matmul(
                        q_ps[:], lhsT=wq_sb[:, kc, c * 128:(c + 1) * 128],
                        rhs=xq_sb[:, kc], start=(kc == 0), stop=(kc == KC - 1))
                z = zp.tile([128, NST], F32, tag="zB")
                nc.vector.tensor_scalar(z[:], q_ps[:], 0.0, EPS, ALU.max, ALU.add)
                y_c = yp.tile([128, NST], F32R, tag="yB")
                nc.scalar.activation(y_c[:], z[:], AF.Square)
                y2 = y2p.tile([128, NST], F32R, tag="y2B")
                nc.scalar.activation(y2[:], y_c[:].bitcast(F32), AF.Square)
                nc.tensor.matmul(sums_ps[:, 0], lhsT=ind_sb[:, hh], rhs=y_c[:],
                                 start=(c == 0), stop=(c == 15))
                nc.tensor.matmul(sums_ps[:, 1], lhsT=ind_sb[:, hh], rhs=y2[:],
                                 start=(c == 0), stop=(c == 15))
                nc.tensor.matmul(sums_ps[:, 2], lhsT=sumk_w[:, c], rhs=y_c[:],
                                 start=(c == 0), stop=(c == 15))
                ys.append(y_c)

            # stats on [8, NST]
            rec2 = stp.tile([8, NST], F32, tag="rec2")
            nc.vector.reciprocal(rec2[:], sums_ps[:, 1])
            rat = stp.tile([8, NST], F32, tag="ratB")
            nc.vector.tensor_mul(rat[:], sums_ps[:, 0], rec2[:])
            fac = stp.tile([8, NST], F32, tag="facB")
            nc.scalar.activation(fac[:], rat[:], AF.Sqrt)
            den = stp.tile([8, NST], F32, tag="den")
            nc.vector.tensor_mul(den[:], sums_ps[:, 2], fac[:])
            nc.vector.tensor_scalar_add(den[:], den[:], eps_sb[:])
            rden = stp.tile([8, NST], F32, tag="rden")
            nc.vector.reciprocal(rden[:], den[:])
            s_sb = stp.tile([8, NST], F32R, tag="sB")
            nc.vector.tensor_mul(s_sb[:], fac[:], rden[:])

            # phi' = y * s (s broadcast across partitions via K=1 matmul)
            phis = []
            for hh in range(8):
                sbc = sbp.tile([128, NST], F32, tag="sbc")
                nc.tensor.matmul(sbc[:], lhsT=ind2_sb[:, hh], rhs=s_sb[:],
                                 start=True, stop=True)
                for mc in range(2):
                    phi_c = php.tile([128, NST], F32R, tag="phB")
                    nc.vector.tensor_mul(phi_c[:], ys[2 * hh + mc][:].bitcast(F32),
                                         sbc[:])
                    phis.append(phi_c)

            # attnT chunks: attnT[(h,dc)] = sum_mc ktv[h,mc,dc]^T phi[(h,mc)] + vssT
            ats = []
            for c in range(16):
                hh, dc = c // 2, c % 2
                at_ps = ap_.tile([128, NST], F32, tag="atps")
                for mc in range(2):
                    nc.tensor.matmul(at_ps[:], lhsT=ktv_sb[:, hh, mc, dc],
                                     rhs=phis[2 * hh + mc][:],
                                     start=(mc == 0), stop=False)
                for kc in range(KC):
                    nc.tensor.matmul(at_ps[:], lhsT=wvm_sb[:, kc, c * 128:(c + 1) * 128],
                                     rhs=xs_sb[:, kc],
                                     start=False, stop=(kc == KC - 1))
                at_sb = atp.tile([128, NST], F32R, tag="atB")
                nc.scalar.copy(at_sb[:], at_ps[:])
                ats.append(at_sb)

            # final projection per 128-node subtile + Lorentz lift
            for sn in range(NST // 128):
                o_ps = op.tile([128, D], F32, tag="ops")
                for c in range(16):
                    nc.tensor.matmul(o_ps[:], lhsT=ats[c][:, sn * 128:(sn + 1) * 128],
                                     rhs=fw_sb[:, c], start=(c == 0), stop=False)
                nc.tensor.matmul(o_ps[:], lhsT=ones_r_sb[:], rhs=fb_sb[:],
                                 start=False, stop=True)
                sq = zp.tile([128, D], F32, tag="sqB")
                ssum = stp.tile([128, 1], F32, tag="ssum")
                nc.scalar.activation(sq[:], o_ps[:], AF.Square,
                                     accum_out=ssum[:])
                tcol = stp.tile([128, 1], F32, tag="tcol")
                nc.scalar.activation(tcol[:], ssum[:], AF.Sqrt, bias=1.0)
                o_sb = obp.tile([128, 257], F32, tag="osb")
                nc.vector.tensor_copy(o_sb[:, 1:257], o_ps[:])
                nc.vector.tensor_copy(o_sb[:, 0:1], tcol[:])
                nc.sync.dma_start(out[nofs + sn * 128: nofs + (sn + 1) * 128, :],
                                  o_sb[:])


def _prep_inputs(query_input, source_input, Wq_w, Wq_b, Wk_w, Wk_b, Wv_w, Wv_b,
                 norm_scale, v_map_w, v_map_b, final_w, final_b):
    def pad_x(x):
        xt = np.zeros((KC * 128, N), np.float32)
        xt[0:257] = x.T
        xt[257] = 1.0
        return xt.reshape(KC, 128, N)

    def pad_w(w_flat, b_flat):
        wt = np.zeros((KC * 128, HD), np.float32)
        wt[0:257] = w_flat.T
        wt[257] = b_flat
        return wt.reshape(KC, 128, HD)

    xq = pad_x(np.asarray(query_input))
    xs = pad_x(np.asarray(source_input))
    wq_h = pad_w(np.asarray(Wq_w).reshape(HD, 257), np.asarray(Wq_b).reshape(HD))
    wk_h = pad_w(np.asarray(Wk_w).reshape(HD, 257), np.asarray(Wk_b).reshape(HD))
    wv_h = pad_w(np.asarray(Wv_w).reshape(HD, 257), np.asarray(Wv_b).reshape(HD))

    vm = np.asarray(v_map_w)
    # wvm_flat[h] = vm @ Wv_w[h]  -> [H, 256, 257]
    wvm_flat = np.einsum('od,hdi->hoi', vm, np.asarray(Wv_w))
    bvm = (np.asarray(Wv_b) @ vm.T + np.asarray(v_map_b)[None, :]).reshape(HD)
    wvm_h = pad_w(wvm_flat.reshape(HD, 257), bvm)

    fw_h = np.ascontiguousarray(np.asarray(final_w).T).reshape(16, 128, D)
    fb_h = np.asarray(final_b).reshape(1, D).astype(np.float32)

    s = abs(float(np.asarray(norm_scale))) + EPS
    eps_eff = EPS * s * s
    cons = np.full((8, 1), eps_eff, np.float32)

    ind = np.zeros((128, 8, 8), np.float32)
    for hh in range(8):
        ind[:, hh, hh] = 1.0
    ind2 = np.zeros((8, 8, 128), np.float32)
    for hh in range(8):
        ind2[hh, hh, :] = 1.0

    common = {
        "wq": wq_h, "wk": wk_h, "wv": wv_h, "wvm": wvm_h,
        "fw": fw_h.astype(np.float32), "fbias": fb_h,
        "ones_r": np.ones((1, 128), np.float32),
        "ones_c": np.ones((128, 8), np.float32),
        "ind": ind, "ind2": ind2, "zt": np.zeros((128, 16, 8), np.float32),
        "cons": cons,
    }
    in_maps = []
    for c in range(NCORES):
        m = dict(common)
        m["xqT"] = np.ascontiguousarray(xq[:, :, c * NCHUNK:(c + 1) * NCHUNK])
        m["xsT"] = np.ascontiguousarray(xs[:, :, c * NCHUNK:(c + 1) * NCHUNK])
        in_maps.append(m)
    return in_maps


def kernel(reps=1, **inputs):
    nc = _build(reps)
    in_maps = _prep_inputs(**inputs)
    res = run_bass_kernel_spmd(nc, in_maps, list(range(NCORES)))
    return np.concatenate([res.results[c]["out"] for c in range(NCORES)], axis=0)



# revision 17
# speedup vs baseline: 577.8280x; 577.8280x over previous
"""Trainium2 Bass kernel for hyperbolic linear-attention transformer layer.

Strategy (v2): the per-rep cost on this stack is dominated by instruction
stream length, so the whole computation lives inside one For_i hardware
loop over reps. Collectives cannot execute inside rolled loops (NRT
requires statically-known collective order), so instead of the classic
data-parallel AllReduce of ktv/sumk partials, every core redundantly
computes the full ktv = phi_k^T v and sumk = sum(phi_k) over ALL
N=32768 nodes (Phase A), keeping the math exact with zero communication.
Phase B (q projection, phi_q, numerator/denominator, v_map path, final
projection, Lorentz lift) runs on the core's 4096-node shard only.

ktv/sumk pass from Phase A to Phase B through SBUF (no DRAM roundtrip).

All matmuls run as float32r (full PE rate at moving-dim>=256).
"""

import os
import numpy as np
import concourse.bass as bass
import concourse.tile as tile
from concourse import bacc, mybir
from concourse.bass_utils import run_bass_kernel_spmd

F32 = mybir.dt.float32
F32R = mybir.dt.float32r
AF = mybir.ActivationFunctionType
ALU = mybir.AluOpType

NCORES = 8
N = 32768
NCHUNK = N // NCORES          # 4096 nodes per core
H = 8
D = 256
HD = H * D                    # 2048
KC = 3                        # contraction chunks: 384 = 3*128 (258 used)
EPS = 1e-6

_CACHE = {}


def _build(reps=1):
    if reps in _CACHE:
        return _CACHE[reps]
    nc = bacc.Bacc("TRN2", target_bir_lowering=False, debug=False,
                   num_devices=NCORES)

    xqT = nc.dram_tensor("xqT", [KC, 128, NCHUNK], F32R, kind="ExternalInput").ap()
    xbT = nc.dram_tensor("xbT", [KC, 128, NCHUNK], F32R, kind="ExternalInput").ap()
    xsT = nc.dram_tensor("xsT", [KC, 128, N], F32R, kind="ExternalInput").ap()
    wq = nc.dram_tensor("wq", [KC, 128, HD], F32R, kind="ExternalInput").ap()
    wk = nc.dram_tensor("wk", [KC, 128, HD], F32R, kind="ExternalInput").ap()
    wv = nc.dram_tensor("wv", [KC, 128, HD], F32R, kind="ExternalInput").ap()
    wvm = nc.dram_tensor("wvm", [KC, 128, HD], F32R, kind="ExternalInput").ap()
    fw = nc.dram_tensor("fw", [16, 128, D], F32R, kind="ExternalInput").ap()
    fbias = nc.dram_tensor("fbias", [1, D], F32R, kind="ExternalInput").ap()
    ones_r = nc.dram_tensor("ones_r", [1, 128], F32R, kind="ExternalInput").ap()
    ones_c = nc.dram_tensor("ones_c", [128, 8], F32R, kind="ExternalInput").ap()
    ind = nc.dram_tensor("ind", [128, 8, 8], F32R, kind="ExternalInput").ap()
    ind2 = nc.dram_tensor("ind2", [8, 8, 128], F32R, kind="ExternalInput").ap()
    cons = nc.dram_tensor("cons", [8, 1], F32, kind="ExternalInput").ap()
    out = nc.dram_tensor("out", [NCHUNK, 257], F32, kind="ExternalOutput").ap()

    with tile.TileContext(nc) as tc:
        _body(nc, tc, reps, xqT, xbT, xsT, wq, wk, wv, wvm, fw, fbias,
              ones_r, ones_c, ind, ind2, cons, out)
    nc.compile()
    _CACHE[reps] = nc
    return nc


def _body(nc, tc, reps, xqT, xbT, xsT, wq, wk, wv, wvm, fw, fbias,
          ones_r, ones_c, ind, ind2, cons, out):
    import contextlib
    stack = contextlib.ExitStack()
    with stack:
        cpool = stack.enter_context(tc.tile_pool(name="const", bufs=1))

        ones_r_sb = cpool.tile([1, 128], F32R)
        nc.sync.dma_start(ones_r_sb[:], ones_r[:])
        ones_c_sb = cpool.tile([128, 8], F32R)
        nc.sync.dma_start(ones_c_sb[:], ones_c[:])
        ind_sb = cpool.tile([128, 8, 8], F32R)
        nc.sync.dma_start(ind_sb[:], ind[:])
        ind2_sb = cpool.tile([8, 8, 128], F32R)
        nc.sync.dma_start(ind2_sb[:], ind2[:])
        fb_sb = cpool.tile([1, D], F32R)
        nc.sync.dma_start(fb_sb[:], fbias[:])
        eps_sb = cpool.tile([8, 1], F32)
        nc.sync.dma_start(eps_sb[:], cons[:])

        # phase A -> phase B carriers (SBUF, rewritten each rep)
        # ktv laid out as lhsT chunks [m_loc, h, mc, dc, d_loc]
        ktv_sb = cpool.tile([128, H, 2, 2, 128], F32R)
        # sumk chunk columns: [128, 16, 8]; chunk c -> column h(c), rest zero
        sumk_w = cpool.tile([128, 16, 8], F32R)

        import contextlib as _ctx
        rep_cm = (tc.For_i(0, reps, name="reploop")
                  if not os.environ.get("KT_PYREPS") else _ctx.nullcontext())
        with rep_cm:
            if not os.environ.get("KT_SKIP_A"):
                _phase_a(nc, tc, xsT, wk, wv, ones_c_sb, ind_sb,
                         ktv_sb, sumk_w)
            if not os.environ.get("KT_SKIP_B"):
                _phase_b(nc, tc, xqT, xbT, wq, wvm, fw, fb_sb,
                         ones_r_sb, ind_sb, ind2_sb, eps_sb,
                         ktv_sb, sumk_w, out)


def _phase_a(nc, tc, xsT, wk, wv, ones_c_sb, ind_sb, ktv_sb, sumk_w):
    import contextlib
    with contextlib.ExitStack() as st:
        apool = st.enter_context(tc.tile_pool(name="accA", bufs=1))
        xp = st.enter_context(tc.tile_pool(name="xA", bufs=3))
        zp = st.enter_context(tc.tile_pool(name="zA", bufs=2))
        yp = st.enter_context(tc.tile_pool(name="yA", bufs=2))
        scrp = st.enter_context(tc.tile_pool(name="scrA", bufs=2))
        stp = st.enter_context(tc.tile_pool(name="stA", bufs=4))
        php = st.enter_context(tc.tile_pool(name="phA", bufs=3))
        vp = st.enter_context(tc.tile_pool(name="vA", bufs=3))
        drp = st.enter_context(tc.tile_pool(name="drA", bufs=2))
        pk = st.enter_context(tc.tile_pool(name="psAk", bufs=1, space="PSUM"))
        pp = st.enter_context(tc.tile_pool(name="psAp", bufs=3, space="PSUM"))
        psk = st.enter_context(tc.tile_pool(name="psAs", bufs=1, space="PSUM"))

        wk_sb = apool.tile([128, KC, HD], F32R)
        nc.sync.dma_start(wk_sb[:], wk.rearrange("c p n -> p c n"))
        wv_sb = apool.tile([128, KC, HD], F32R)
        nc.sync.dma_start(wv_sb[:], wv.rearrange("c p n -> p c n"))
        sumk_acc = apool.tile([128, HD], F32R)

        def ktv_mms(ktv_ps, phi, v_sb, first, last):
            # ktv[h][m,d] += phi[:, h*256+mc*128]^T v[:, h*256:+256]
            for hh in range(4):
                for mc in range(2):
                    nc.tensor.matmul(
                        ktv_ps[:, hh, mc * 256: mc * 256 + 256],
                        lhsT=phi[:, hh * 256 + mc * 128: hh * 256 + mc * 128 + 128],
                        rhs=v_sb[:, hh * 256: hh * 256 + 256],
                        start=first, stop=last)

        ntiles = int(os.environ.get("KT_NTILES", N // 128))
        for g in range(2):
            gofs = g * 1024
            ktv_ps = pk.tile([128, 4, 512], F32)
            prev = None
            for t in range(ntiles):
                xs_sb = xp.tile([128, KC, 128], F32R, tag="xs")
                nc.sync.dma_start(
                    xs_sb[:],
                    xsT[:, :, t * 128:(t + 1) * 128].rearrange("c p n -> p c n"))

                ks_ps = []
                vs_ps = []
                for blk in range(2):
                    kp_t = pp.tile([128, 512], F32, tag="projA")
                    for c in range(KC):
                        nc.tensor.matmul(
                            kp_t[:], lhsT=xs_sb[:, c],
                            rhs=wk_sb[:, c, gofs + blk * 512: gofs + blk * 512 + 512],
                            start=(c == 0), stop=(c == KC - 1))
                    ks_ps.append(kp_t)
                for blk in range(2):
                    vp_t = pp.tile([128, 512], F32, tag="projA")
                    for c in range(KC):
                        nc.tensor.matmul(
                            vp_t[:], lhsT=xs_sb[:, c],
                            rhs=wv_sb[:, c, gofs + blk * 512: gofs + blk * 512 + 512],
                            start=(c == 0), stop=(c == KC - 1))
                    vs_ps.append(vp_t)

                # lagged ktv for the previous tile: keeps the PE busy while
                # this tile's z/y/phi chain runs on DVE/ACT
                if prev is not None:
                    ktv_mms(ktv_ps, prev[0], prev[1], prev[2] == 0, False)

                # z = relu(ks) + eps
                z = zp.tile([128, 1024], F32, tag="z")
                for blk in range(2):
                    nc.vector.tensor_scalar(
                        z[:, blk * 512:(blk + 1) * 512], ks_ps[blk][:],
                        0.0, EPS, ALU.max, ALU.add)
                # v copy to SBUF (frees psum quickly)
                v_sb = vp.tile([128, 1024], F32R, tag="v")
                for blk in range(2):
                    nc.scalar.copy(v_sb[:, blk * 512:(blk + 1) * 512], vs_ps[blk][:])

                # y = z^2 with per-head accumulated sums
                y = yp.tile([128, 1024], F32R, tag="y")
                sy = stp.tile([128, 4], F32, tag="sy")
                sy2 = stp.tile([128, 4], F32, tag="sy2")
                for hh in range(4):
                    sl = slice(hh * 256, hh * 256 + 256)
                    nc.scalar.activation(y[:, sl], z[:, sl], AF.Square,
                                         accum_out=sy[:, hh:hh + 1])
                for hh in range(4):
                    sl = slice(hh * 256, hh * 256 + 256)
                    scr = scrp.tile([128, 256], F32, tag="y2scr")
                    nc.scalar.activation(scr[:], y[:, sl].bitcast(F32), AF.Square,
                                         accum_out=sy2[:, hh:hh + 1])
                # factor = sqrt(sy / sy2)
                rec = stp.tile([128, 4], F32, tag="rec")
                nc.vector.reciprocal(rec[:], sy2[:])
                rat = stp.tile([128, 4], F32, tag="rat")
                nc.vector.tensor_mul(rat[:], sy[:], rec[:])
                fac = stp.tile([128, 4], F32, tag="fac")
                nc.scalar.activation(fac[:], rat[:], AF.Sqrt)

                phi = php.tile([128, 1024], F32R, tag="phi")
                for hh in range(4):
                    sl = slice(hh * 256, hh * 256 + 256)
                    nc.vector.tensor_scalar_mul(phi[:, sl], y[:, sl].bitcast(F32),
                                                fac[:, hh:hh + 1])
                # sumk accumulation
                dst = sumk_acc[:, gofs:gofs + 1024]
                if t == 0:
                    nc.scalar.copy(dst, phi[:].bitcast(F32))
                else:
                    nc.vector.tensor_add(dst, dst.bitcast(F32), phi[:].bitcast(F32))

                prev = (phi, v_sb, t)

            # tail: ktv for the final tile closes the accumulation group
            ktv_mms(ktv_ps, prev[0], prev[1], prev[2] == 0, True)

            # drain ktv psum for this head group straight into lhsT layout:
            # psum free layout per head is (mc, dc, d_loc) = (2, 2, 128)
            for hh in range(4):
                nc.scalar.copy(
                    ktv_sb[:, g * 4 + hh].rearrange("p mc dc dl -> p (mc dc dl)"),
                    ktv_ps[:, hh])

        # sumk partition-reduction: [128, 2048] -> [1, 2048]
        srow = apool.tile([1, HD], F32R)
        for blk in range(4):
            scr = psk.tile([128, 512], F32, tag="pscr")
            nc.tensor.matmul(
                scr[0:8, :], lhsT=ones_c_sb[:],
                rhs=sumk_acc[:, blk * 512:(blk + 1) * 512],
                start=True, stop=True)
            nc.scalar.copy(srow[:, blk * 512:(blk + 1) * 512], scr[0:1, :])

        # transpose each 128-chunk of srow into sumk_w[:, c, :] (col h(c)=sumk,
        # others zero): out[128, 8] = srow_chunk^T (K=1) @ e_h row
        for c in range(16):
            hh = c // 2
            scr = psk.tile([128, 512], F32, tag="pscr")
            nc.tensor.matmul(scr[:, 0:8], lhsT=srow[:, c * 128:(c + 1) * 128],
                             rhs=ind_sb[0:1, hh, :], start=True, stop=True)
            nc.scalar.copy(sumk_w[:, c], scr[:, 0:8])


def _phase_b(nc, tc, xqT, xbT, wq, wvm, fw, fb_sb, ones_r_sb,
             ind_sb, ind2_sb, eps_sb, ktv_sb, sumk_w, out):
    import contextlib
    with contextlib.ExitStack() as st:
        wpool = st.enter_context(tc.tile_pool(name="wB", bufs=1))
        xp = st.enter_context(tc.tile_pool(name="xB", bufs=2))
        zp = st.enter_context(tc.tile_pool(name="zB", bufs=3))
        yp = st.enter_context(tc.tile_pool(name="yB", bufs=17))
        y2p = st.enter_context(tc.tile_pool(name="y2B", bufs=3))
        stp = st.enter_context(tc.tile_pool(name="stB", bufs=2))
        php = st.enter_context(tc.tile_pool(name="phB", bufs=17))
        atp = st.enter_context(tc.tile_pool(name="atB", bufs=17))
        obp = st.enter_context(tc.tile_pool(name="oB", bufs=3))
        qp = st.enter_context(tc.tile_pool(name="psBq", bufs=2, space="PSUM"))
        sump = st.enter_context(tc.tile_pool(name="psBs", bufs=1, space="PSUM"))
        sbp = st.enter_context(tc.tile_pool(name="psBb", bufs=1, space="PSUM"))
        ap_ = st.enter_context(tc.tile_pool(name="psBa", bufs=2, space="PSUM"))
        op = st.enter_context(tc.tile_pool(name="psBo", bufs=1, space="PSUM"))

        wq_sb = wpool.tile([128, KC, HD], F32R)
        nc.sync.dma_start(wq_sb[:], wq.rearrange("c p n -> p c n"))
        wvm_sb = wpool.tile([128, KC, HD], F32R)
        nc.sync.dma_start(wvm_sb[:], wvm.rearrange("c p n -> p c n"))
        fw_sb = wpool.tile([128, 16, D], F32R)
        nc.sync.dma_start(fw_sb[:], fw.rearrange("c p n -> p c n"))

        NST = 256                      # supertile node count
        nst = int(os.environ.get("KT_NST", NCHUNK // NST))
        for stx in range(nst):
            nofs = stx * NST
            xq_sb = xp.tile([128, KC, NST], F32R, tag="xq")
            nc.sync.dma_start(
                xq_sb[:], xqT[:, :, nofs:nofs + NST].rearrange("c p n -> p c n"))
            xs_sb = xp.tile([128, KC, NST], F32R, tag="xsB")
            nc.sync.dma_start(
                xs_sb[:], xbT[:, :, nofs:nofs + NST].rearrange("c p n -> p c n"))

            sums_ps = sump.tile([8, 3, NST], F32, tag="sums")
            ys = []
            for c in range(16):
                hh = c // 2
                q_ps = qp.tile([128, NST], F32, tag="qps")
                for kc in range(KC):
                    nc.tensor.matmul(
                        q_ps[:], lhsT=wq_sb[:, kc, c * 128:(c + 1) * 128],
                        rhs=xq_sb[:, kc], start=(kc == 0), stop=(kc == KC - 1))
                z = zp.tile([128, NST], F32, tag="zB")
                nc.vector.tensor_scalar(z[:], q_ps[:], 0.0, EPS, ALU.max, ALU.add)
                y_c = yp.tile([128, NST], F32R, tag="yB")
                nc.scalar.activation(y_c[:], z[:], AF.Square)
                y2 = y2p.tile([128, NST], F32R, tag="y2B")
                nc.scalar.activation(y2[:], y_c[:].bitcast(F32), AF.Square)
                nc.tensor.matmul(sums_ps[:, 0], lhsT=ind_sb[:, hh], rhs=y_c[:],
                                 start=(c == 0), stop=(c == 15))
                nc.tensor.matmul(sums_ps[:, 1], lhsT=ind_sb[:, hh], rhs=y2[:],
                                 start=(c == 0), stop=(c == 15))
                nc.tensor.matmul(sums_ps[:, 2], lhsT=sumk_w[:, c], rhs=y_c[:],
                                 start=(c == 0), stop=(c == 15))
                ys.append(y_c)

            # stats on [8, NST]
            rec2 = stp.tile([8, NST], F32, tag="rec2")
            nc.vector.reciprocal(rec2[:], sums_ps[:, 1])
            rat = stp.tile([8, NST], F32, tag="ratB")
            nc.vector.tensor_mul(rat[:], sums_ps[:, 0], rec2[:])
            fac = stp.tile([8, NST], F32, tag="facB")
            nc.scalar.activation(fac[:], rat[:], AF.Sqrt)
            den = stp.tile([8, NST], F32, tag="den")
            nc.vector.tensor_mul(den[:], sums_ps[:, 2], fac[:])
            nc.vector.tensor_scalar_add(den[:], den[:], eps_sb[:])
            rden = stp.tile([8, NST], F32, tag="rden")
            nc.vector.reciprocal(rden[:], den[:])
            s_sb = stp.tile([8, NST], F32R, tag="sB")
            nc.vector.tensor_mul(s_sb[:], fac[:], rden[:])

            # phi' = y * s (s broadcast across partitions via K=1 matmul)
            phis = []
            for hh in range(8):
                sbc = sbp.tile([128, NST], F32, tag="sbc")
                nc.tensor.matmul(sbc[:], lhsT=ind2_sb[:, hh], rhs=s_sb[:],
                                 start=True, stop=True)
                for mc in range(2):
                    phi_c = php.tile([128, NST], F32R, tag="phB")
                    nc.vector.tensor_mul(phi_c[:], ys[2 * hh + mc][:].bitcast(F32),
                                         sbc[:])
                    phis.append(phi_c)

            # attnT chunks: attnT[(h,dc)] = sum_mc ktv[h,mc,dc]^T phi[(h,mc)] + vssT
            ats = []
            for c in range(16):
                hh, dc = c // 2, c % 2
                at_ps = ap_.tile([128, NST], F32, tag="atps")
                for mc in range(2):
                    nc.tensor.matmul(at_ps[:], lhsT=ktv_sb[:, hh, mc, dc],
                                     rhs=phis[2 * hh + mc][:],
                                     start=(mc == 0), stop=False)
                for kc in range(KC):
                    nc.tensor.matmul(at_ps[:], lhsT=wvm_sb[:, kc, c * 128:(c + 1) * 128],
                                     rhs=xs_sb[:, kc],
                                     start=False, stop=(kc == KC - 1))
                at_sb = atp.tile([128, NST], F32R, tag="atB")
                nc.scalar.copy(at_sb[:], at_ps[:])
                ats.append(at_sb)

            # final projection per 128-node subtile + Lorentz lift
            for sn in range(NST // 128):
                o_ps = op.tile([128, D], F32, tag="ops")
                for c in range(16):
                    nc.tensor.matmul(o_ps[:], lhsT=ats[c][:, sn * 128:(sn + 1) * 128],
                                     rhs=fw_sb[:, c], start=(c == 0), stop=False)
                nc.tensor.matmul(o_ps[:], lhsT=ones_r_sb[:], rhs=fb_sb[:],
                                 start=False, stop=True)
                sq = zp.tile([128, D], F32, tag="sqB")
                ssum = stp.tile([128, 1], F32, tag="ssum")
                nc.scalar.activation(sq[:], o_ps[:], AF.Square,
                                     accum_out=ssum[:])
                tcol = stp.tile([128, 1], F32, tag="tcol")
                nc.scalar.activation(tcol[:], ssum[:], AF.Sqrt, bias=1.0)
                o_sb = obp.tile([128, 257], F32, tag="osb")
                nc.vector.tensor_copy(o_sb[:, 1:257], o_ps[:])
                nc.vector.tensor_copy(o_sb[:, 0:1], tcol[:])
                nc.sync.dma_start(out[nofs + sn * 128: nofs + (sn + 1) * 128, :],
                                  o_sb[:])


def _prep_inputs(query_input, source_input, Wq_w, Wq_b, Wk_w, Wk_b, Wv_w, Wv_b,
                 norm_scale, v_map_w, v_map_b, final_w, final_b):
    def pad_x(x):
        xt = np.zeros((KC * 128, N), np.float32)
        xt[0:257] = x.T
        xt[257] = 1.0
        return xt.reshape(KC, 128, N)

    def pad_w(w_flat, b_flat):
        wt = np.zeros((KC * 128, HD), np.float32)
        wt[0:257] = w_flat.T
        wt[257] = b_flat
        return wt.reshape(KC, 128, HD)

    xq = pad_x(np.asarray(query_input))
    xs = pad_x(np.asarray(source_input))
    wq_h = pad_w(np.asarray(Wq_w).reshape(HD, 257), np.asarray(Wq_b).reshape(HD))
    wk_h = pad_w(np.asarray(Wk_w).reshape(HD, 257), np.asarray(Wk_b).reshape(HD))
    wv_h = pad_w(np.asarray(Wv_w).reshape(HD, 257), np.asarray(Wv_b).reshape(HD))

    vm = np.asarray(v_map_w)
    # wvm_flat[h] = vm @ Wv_w[h]  -> [H, 256, 257]
    wvm_flat = np.einsum('od,hdi->hoi', vm, np.asarray(Wv_w))
    bvm = (np.asarray(Wv_b) @ vm.T + np.asarray(v_map_b)[None, :]).reshape(HD)
    wvm_h = pad_w(wvm_flat.reshape(HD, 257), bvm)

    fw_h = np.ascontiguousarray(np.asarray(final_w).T).reshape(16, 128, D)
    fb_h = np.asarray(final_b).reshape(1, D).astype(np.float32)

    s = abs(float(np.asarray(norm_scale))) + EPS
    eps_eff = EPS * s * s
    cons = np.full((8, 1), eps_eff, np.float32)

    ind = np.zeros((128, 8, 8), np.float32)
    for hh in range(8):
        ind[:, hh, hh] = 1.0
    ind2 = np.zeros((8, 8, 128), np.float32)
    for hh in range(8):
        ind2[hh, hh, :] = 1.0

    common = {
        "xsT": xs,
        "wq": wq_h, "wk": wk_h, "wv": wv_h, "wvm": wvm_h,
        "fw": fw_h.astype(np.float32), "fbias": fb_h,
        "ones_r": np.ones((1, 128), np.float32),
        "ones_c": np.ones((128, 8), np.float32),
        "ind": ind, "ind2": ind2,
        "cons": cons,
    }
    in_maps = []
    for c in range(NCORES):
        m = dict(common)
        m["xqT"] = np.ascontiguousarray(xq[:, :, c * NCHUNK:(c + 1) * NCHUNK])
        m["xbT"] = np.ascontiguousarray(xs[:, :, c * NCHUNK:(c + 1) * NCHUNK])
        in_maps.append(m)
    return in_maps


def kernel(reps=1, **inputs):
    nc = _build(reps)
    in_maps = _prep_inputs(**inputs)
    res = run_bass_kernel_spmd(nc, in_maps, list(range(NCORES)))
    return np.concatenate([res.results[c]["out"] for c in range(NCORES)], axis=0)
